# revision 13
# baseline (speedup 1.0000x reference)
"""DigitCaps (capsule routing) Trainium2 Bass kernel, v2.

u [512, 1152, 8] f32, W [1, 1152, 10, 16, 8] f32 -> v [512, 10, 16] f32
(3 dynamic-routing iterations, softmax over 10 classes).

Data-parallel: batch 64 per core x 8 cores. Within a core the batch is
split into two 32-wide chunks whose phases are emitted interleaved
(TA(j,h0), TA(j,h1), SMXC(j,h0), TA(j+1,h0), SMXC(j,h1), ...) so the
ACT-engine T-evacuations of one chunk overlap the DVE-heavy xc phase of
the other.

Engine assignment (per routing iteration):
  PE:   T[b,i,c,k] = sum_d W[i,c,d,k] v[b,c,d]  (block-diag vT rhs)
        s[b,c,d]   = sum_{ik} W xc               (per-class col-groups)
  ACT:  PSUM->SBUF T evacuations in k-pairs (Copy), exp(scale=1/WSCALE).
        Only Copy/Exp/Identity -> a single act-function table load.
  DVE:  P = T*u pair-mults, tree level 1, reciprocal, uTs, all xc,
        sqrt-free squash (Babylonian iteration on [32,10] tiles).
  Pool: tree levels 2+3 (+logit accumulate), softmax partial sums and
        denominator, memsets.

Routing weights (wt/wtb, used only for the agreement logits) are fp8e4
scaled by WSCALE; exp(L/WSCALE) compensates. s-path weights (wsk) stay
bf16: output precision depends on them directly.

Layouts (per core):
  i: block g = i//128 (9 blocks), partition r = i%128
  class c = 2p+ch, pass p in [0,5), parity ch in {0,1}
  logits/exp: [r, p, (g, ch, b)]  (b covers the full 64; chunks slice b)
"""

import numpy as np

N_CORES = 8
B_PER = 64
BH = 32          # chunk width
I_CAPS = 1152
K_DIM = 8
C_CLS = 10
D_DIM = 16
NG = I_CAPS // 128  # 9
EPS = 1e-8
WSCALE = 64.0    # fp8 pre-scale of the routing weights
USE_FP8_WT = True

_CACHE = {}


def _build():
    import concourse.bass as bass
    import concourse.mybir as mybir
    from concourse import tile, bacc

    f32 = mybir.dt.float32
    bf16 = mybir.dt.bfloat16
    f8 = mybir.dt.float8e4
    wdt = f8 if USE_FP8_WT else bf16
    wscale = WSCALE if USE_FP8_WT else 1.0
    AF = mybir.ActivationFunctionType
    OP = mybir.AluOpType

    nc = bacc.Bacc()
    uTk_in = nc.dram_tensor(
        "uTk_h", [128, K_DIM, NG, B_PER], bf16, kind="ExternalInput"
    )
    u8_in = nc.dram_tensor(
        "u8_h", [128, K_DIM, NG, B_PER], wdt, kind="ExternalInput"
    )
    wsk_in = nc.dram_tensor(
        "wsk_h", [128, K_DIM, NG, C_CLS, D_DIM], bf16, kind="ExternalInput"
    )
    wsk8_in = nc.dram_tensor(
        "wsk8_h", [128, K_DIM, NG, C_CLS, D_DIM], wdt, kind="ExternalInput"
    )
    wt_in = nc.dram_tensor("wt_h", [128, K_DIM, I_CAPS], wdt, kind="ExternalInput")
    wtb_in = nc.dram_tensor("wtb_h", [32, K_DIM, I_CAPS], wdt, kind="ExternalInput")
    eyeb_in = nc.dram_tensor("eyeb", [128, 128], bf16, kind="ExternalInput")
    v_out = nc.dram_tensor("v", [B_PER, C_CLS, D_DIM], f32, kind="ExternalOutput")

    with tile.TileContext(nc) as tc:
        perm = tc.alloc_tile_pool(name="perm", bufs=1)
        Wsk = perm.tile([128, K_DIM, NG, C_CLS, D_DIM], bf16)  # [r,(k,g,c,d)]
        Wsk8 = perm.tile([128, K_DIM, NG, C_CLS, D_DIM], wdt)  # fp8 copy for s0
        WT = perm.tile([128, K_DIM, I_CAPS], wdt)   # rows 16c+d (classes 0-7)
        WTB = perm.tile([128, K_DIM, I_CAPS], wdt)  # rows 96:128: classes 8,9
        uTk = perm.tile([128, K_DIM, NG, B_PER], bf16)      # u[b, 128g+r, k]
        U8 = perm.tile([128, K_DIM, NG, B_PER], wdt)        # fp8 copy for s0
        L = perm.tile([128, 5, NG, 2, B_PER], bf16, name="Lt")    # logits i-major
        cE = perm.tile([128, 5, NG, 2, B_PER], bf16, name="cEt")  # exp(L/WSCALE)
        recT = perm.tile([128, NG, B_PER], bf16, name="recTt")    # 1/den i-major
        eye_sb = perm.tile([128, 128], bf16)

        psS = tc.alloc_tile_pool(name="psS", bufs=1, space="PSUM")
        psT = tc.alloc_tile_pool(name="psT", bufs=2, space="PSUM")
        psV = tc.alloc_tile_pool(name="psV", bufs=1, space="PSUM")

        def pv_tile():
            return psV.tile([128, 80], bf16, tag="pv", name="pvt")

        # per-chunk small tiles (double-buffered via pools)
        smp = tc.alloc_tile_pool(name="smp", bufs=2)
        itp = tc.alloc_tile_pool(name="itp", bufs=2)

        # ---------------- input DMA (ordered for the critical path) ----
        # s0 runs on fp8 copies of u and wsk so it only waits for ~2.1MB
        nc.sync.dma_start(U8[:], u8_in[:])
        nc.sync.dma_start(Wsk8[:, 0:4], wsk8_in[:, 0:4])
        nc.sync.dma_start(eye_sb[:], eyeb_in[:])
        nc.sync.dma_start(Wsk8[:, 4:8], wsk8_in[:, 4:8])
        nc.sync.dma_start(uTk[:, 0:4], uTk_in[:, 0:4])
        nc.sync.dma_start(uTk[:, 4:8], uTk_in[:, 4:8])
        for kq in range(4):  # wt in k-quarters so pass 0 can start early
            nc.sync.dma_start(
                WT[:, 2 * kq : 2 * kq + 2, :], wt_in[:, 2 * kq : 2 * kq + 2, :]
            )
        nc.sync.dma_start(WTB[96:128, :, :], wtb_in[:])
        nc.sync.dma_start(Wsk[:, 0:4], wsk_in[:, 0:4])
        nc.sync.dma_start(Wsk[:, 4:8], wsk_in[:, 4:8])

        # ---------------- helpers ----------------

        def s0_phase(h):
            """s for iteration 0 (uniform routing): two class-half matmuls
            with W as the stationary side -> [80=(c5,d16), 32] psum, then
            transpose back to [32, (c,d)].  Returns s_sb [32,160] f32."""
            s_sb = smp.tile([32, C_CLS, D_DIM], f32, tag="ssb", name=f"s0sb{h}")
            for half in range(2):
                # borrow the (idle at this point) double-buffered psT tiles
                # so the four s0 groups pipeline instead of serializing on
                # the single psS buffer
                ps_t = psT.tile(
                    [128, 2, NG, 2, BH], f32, tag="pt", name=f"ps0_{h}{half}"
                )
                ps = ps_t[0:80, 0, 0, 0, :]
                n = 0
                for k in range(K_DIM):
                    for g in range(NG):
                        nc.tensor.matmul(
                            ps,
                            Wsk8[:, k, g, 5 * half : 5 * half + 5, :].rearrange(
                                "r c d -> r (c d)"
                            ),
                            U8[:, k, g, BH * h : BH * h + BH],
                            start=(n == 0),
                            stop=(n == K_DIM * NG - 1),
                        )
                        n += 1
                s0T = smp.tile([80, BH], bf16, tag="s0T")
                nc.scalar.activation(
                    s0T[:], ps, AF.Copy, scale=0.1 / wscale
                )
                pv = pv_tile()[0:32, 0:80]
                nc.tensor.transpose(pv, s0T[:], eye_sb[0:80, 0:80])
                nc.scalar.copy(
                    s_sb[:, 5 * half : 5 * half + 5, :].rearrange(
                        "b c d -> b (c d)"
                    ),
                    pv,
                )
            return s_sb

        def squash(h, s_sb, last):
            """v = s * sqrt(n2)/(1+n2) per class (equals the reference's
            n2/((1+n2)(sqrt(n2)+EPS)) up to EPS).  sqrt via 4 Babylonian
            iterations on DVE keeps the ACT engine on one function table."""
            sq = smp.tile([32, C_CLS, D_DIM], f32, tag="sqq")
            n2 = smp.tile([32, C_CLS], f32, tag="sn2")
            n2h = smp.tile([32, C_CLS], f32, tag="sn2h")
            x = smp.tile([32, C_CLS], f32, tag="sxx")
            rx = smp.tile([32, C_CLS], f32, tag="srx")
            qx = smp.tile([32, C_CLS], f32, tag="sqx")
            t1 = smp.tile([32, C_CLS], f32, tag="st1")
            r1 = smp.tile([32, C_CLS], f32, tag="sr1")
            fac = smp.tile([32, C_CLS], f32, tag="sfc")
            v_sb = smp.tile([32, C_CLS, D_DIM], f32, tag="svb", name=f"vsb{h}")
            nc.vector.tensor_tensor(sq[:], s_sb[:], s_sb[:], OP.mult)
            nc.vector.reduce_sum(n2[:], sq[:], axis=mybir.AxisListType.X)
            nc.vector.tensor_scalar_add(t1[:], n2[:], 1.0)
            nc.vector.reciprocal(r1[:], t1[:])
            # Babylonian sqrt(n2): x' = 0.5*x + (0.5*n2)/x, seed (1+n2)/2
            nc.vector.tensor_scalar_mul(n2h[:], n2[:], 0.5)
            # routing-only squashes tolerate a coarser sqrt (like the fp8
            # routing weights): minimax linear seed + 1 Newton (~1% err).
            # The two output squashes keep the robust seed + 4 iterations.
            if last:
                nc.vector.tensor_scalar_mul(x[:], t1[:], 0.5)
            else:
                nc.vector.tensor_scalar(
                    x[:], n2[:], 0.69, 0.276, OP.mult, OP.add
                )
            for _ in range(4 if last else 1):
                nc.vector.reciprocal(rx[:], x[:])
                nc.vector.tensor_tensor(qx[:], n2h[:], rx[:], OP.mult)
                nc.vector.scalar_tensor_tensor(
                    x[:], x[:], 0.5, qx[:], OP.mult, OP.add
                )
            nc.vector.tensor_tensor(fac[:], x[:], r1[:], OP.mult)
            nc.vector.tensor_tensor(
                v_sb[:],
                s_sb[:],
                fac[:].rearrange("b c -> b c ()").to_broadcast(
                    (32, C_CLS, D_DIM)
                ),
                OP.mult,
            )
            if last:
                nc.sync.dma_start(v_out[BH * h : BH * h + BH], v_sb[:])
                return None
            return v_sb

        def build_vT(h, v_sb, on_act=True):
            """vT [128=(c8,d16), 64=(ch,b)] block-diag for classes 0-7 and
            vT4 rows 96:128 for classes 8,9.  on_act=False keeps the copies
            off the ACT queue (used in the s0 era, where ACT must move on to
            the first TA's evacuations without waiting for this chunk)."""
            cp = nc.scalar.copy if on_act else nc.vector.tensor_copy
            in2 = smp.tile([64, 128], bf16, tag="in2")
            in2b = smp.tile([64, 32], bf16, tag="in2b", name=f"i2b{h}")
            nc.gpsimd.memset(in2[:], 0.0)
            i2v = in2[:].rearrange("q (c d) -> q c d", d=D_DIM)
            cp(i2v[0:32, 0::2, :], v_sb[:, 0:8:2, :])
            cp(i2v[32:64, 1::2, :], v_sb[:, 1:8:2, :])
            nc.gpsimd.memset(in2b[:], 0.0)
            cp(in2b[0:32, 0:16], v_sb[:, 8, :])
            cp(in2b[32:64, 16:32], v_sb[:, 9, :])
            vTt = smp.tile([128, 64], bf16, tag="vTt", name=f"vT{h}")
            vT4t = smp.tile([128, 64], bf16, tag="vT4t", name=f"vT4{h}")
            pv = pv_tile()[:, 0:64]
            nc.tensor.transpose(pv, in2[:], eye_sb[0:64, 0:64])
            cp(vTt[:], pv)
            pv4 = pv_tile()[0:32, 0:64]
            nc.tensor.transpose(pv4, in2b[:], eye_sb[0:64, 0:64])
            cp(vT4t[96:128, :], pv4)
            return vTt, vT4t

        def ta_pass_fns(j, h, vTt, vT4t, bts, last_ta=False):
            """One closure per pass: T = W vT per (pass, k-pair); evac pairs
            on ACT; P = T*u pair-mults + tree on DVE; logit update on Pool;
            exp on ACT; softmax partial sums on Pool."""

            def one_pass(p):
                if p < 4:
                    lhsW, row0, vrhs = WT, 32 * p, vTt[32 * p : 32 * p + 32, :]
                else:
                    lhsW, row0, vrhs = WTB, 96, vT4t[96:128, :]
                Tp = itp.tile([128, K_DIM, NG, 2, BH], bf16, tag="tp", bufs=3)
                for kp in range(4):
                    pt = psT.tile([128, 2, NG, 2, BH], f32, tag="pt")
                    for kk in range(2):
                        k = 2 * kp + kk
                        for g in range(NG):
                            nc.tensor.matmul(
                                pt[:, kk, g, :, :].rearrange("r c b -> r (c b)"),
                                lhsW[row0 : row0 + 32, k, 128 * g : 128 * (g + 1)],
                                vrhs,
                                start=True,
                                stop=True,
                                tile_position=(row0, 0),
                            )
                    nc.scalar.copy(
                        Tp[:, 2 * kp : 2 * kp + 2].rearrange(
                            "r k g c b -> r (k g c b)"
                        ),
                        pt[:].rearrange("r k g c b -> r (k g c b)"),
                    )
                # P = T * u  (u broadcast over the class parity)
                P = itp.tile([128, K_DIM, NG, 2, BH], bf16, tag="pp")
                for kp in range(4):
                    k0 = 2 * kp
                    nc.vector.tensor_tensor(
                        P[:, k0 : k0 + 2],
                        Tp[:, k0 : k0 + 2],
                        uTk[:, k0 : k0 + 2, :, BH * h : BH * h + BH].rearrange(
                            "r k g b -> r k g () b"
                        ).to_broadcast((128, 2, NG, 2, BH)),
                        OP.mult,
                    )
                t1a = itp.tile([128, 4, NG, 2, BH], bf16, tag="t4")
                nc.vector.tensor_tensor(t1a[:], P[:, 0:4], P[:, 4:8], OP.add)
                t2a = itp.tile([128, 2, NG, 2, BH], bf16, tag="t2")
                nc.vector.tensor_tensor(t2a[:], t1a[:, 0:2], t1a[:, 2:4], OP.add)
                Lv = L[:, p, :, :, BH * h : BH * h + BH]
                # the final TA's last pass feeds the exposed smxc tail:
                # keep its logit chain on DVE to skip two Pool round-trips
                eng = nc.vector if (last_ta and p == 4) else nc.gpsimd
                if j == 0:
                    eng.tensor_tensor(Lv, t2a[:, 0], t2a[:, 1], OP.add)
                else:
                    Lp = itp.tile([128, NG, 2, BH], bf16, tag="lp")
                    eng.tensor_tensor(Lp[:], t2a[:, 0], t2a[:, 1], OP.add)
                    eng.tensor_tensor(Lv, Lp[:], Lv, OP.add)

            def exp_pass(p):
                # deferred one pass so the ACT evac stream is never blocked
                # behind a Pool-fed logit update
                cEv = cE[:, p, :, :, BH * h : BH * h + BH]
                Lv = L[:, p, :, :, BH * h : BH * h + BH]
                nc.scalar.activation(cEv, Lv, AF.Exp, scale=1.0 / wscale)
                beng = nc.vector if (last_ta and p == 4) else nc.gpsimd
                beng.tensor_tensor(
                    bts[p][:], cEv[:, :, 0, :], cEv[:, :, 1, :], OP.add
                )
                if p == 1:
                    nc.gpsimd.tensor_tensor(bts[0][:], bts[0][:], bts[1][:], OP.add)
                elif p == 3:
                    nc.gpsimd.tensor_tensor(bts[2][:], bts[2][:], bts[3][:], OP.add)
                    # den02 = b0+b1+b2+b3, off the phase-boundary path
                    nc.gpsimd.tensor_tensor(bts[0][:], bts[0][:], bts[2][:], OP.add)

            def staged(p):
                # exp deferred two passes: the Pool queue now leads with the
                # next smxc's precomputed xc pair, so the logit chain needs
                # deeper slack before ACT consumes it
                one_pass(p)
                if p > 1:
                    exp_pass(p - 2)
                if p == 4:
                    if last_ta:
                        # exp(4)'s logit comes off DVE (fast); exp(3)'s off
                        # Pool — fire the fast one first so the exposed
                        # head's denominator starts sooner
                        exp_pass(4)
                        exp_pass(3)
                    else:
                        exp_pass(3)
                        exp_pass(4)

            return [lambda p=p: staged(p) for p in range(5)]

        def smxc_fns(j, h, bts, last, exposed=False):
            """Pieces of the softmax/xc/s phase: [head, xc0..xc4, tail].
            The pieces are interleaved between the passes of the other
            chunk's TA so DVE always has ready work while ACT evacuates."""
            state = {}

            def head():
                den = smp.tile([128, NG, BH], bf16, tag="smd")
                # at the exposed boundary there is no big parallel DVE tail,
                # so the denominator's Pool hops sit on the critical path:
                # run it on DVE there (latency over balance)
                deng = nc.vector if exposed else nc.gpsimd
                deng.tensor_tensor(den[:], bts[0][:], bts[4][:], OP.add)
                with nc.allow_low_precision(
                    reason="softmax reciprocal to bf16 ok"
                ):
                    nc.vector.reciprocal(
                        recT[:, :, BH * h : BH * h + BH],
                        den[:],
                    )
                uTs = itp.tile([128, K_DIM, NG, BH], bf16, tag="uts")
                nc.vector.tensor_tensor(
                    uTs[:],
                    uTk[:, :, :, BH * h : BH * h + BH],
                    recT[:, :, BH * h : BH * h + BH].rearrange(
                        "r g b -> r () g b"
                    ).to_broadcast((128, K_DIM, NG, BH)),
                    OP.mult,
                )
                state["uTs"] = uTs
                state["ps"] = psS.tile(
                    [80, 160], f32, tag="psu", name=f"pss{j}{h}"
                )
                # precompute the whole last class-pair's xc on Pool, at the
                # head: it finishes long before the PE queue reaches its
                # matmuls, and cuts the DVE xc tail by one pair
                xc89 = itp.tile([128, K_DIM, NG, 2, BH], bf16, tag="xc9")
                nc.gpsimd.tensor_tensor(
                    xc89[:],
                    uTs[:].rearrange("r k g b -> r k g () b").to_broadcast(
                        (128, K_DIM, NG, 2, BH)
                    ),
                    cE[:, 4, :, :, BH * h : BH * h + BH].rearrange(
                        "r g c b -> r () g c b"
                    ).to_broadcast((128, K_DIM, NG, 2, BH)),
                    OP.mult,
                )
                state["xc89"] = xc89

            def xc_piece(p):
                uTs, ps = state["uTs"], state["ps"]
                if p == 4:
                    xc = state["xc89"]
                else:
                    xc = itp.tile([128, K_DIM, NG, 2, BH], bf16, tag="xc")
                    nc.vector.tensor_tensor(
                        xc[:],
                        uTs[:].rearrange("r k g b -> r k g () b").to_broadcast(
                            (128, K_DIM, NG, 2, BH)
                        ),
                        cE[:, p, :, :, BH * h : BH * h + BH].rearrange(
                            "r g c b -> r () g c b"
                        ).to_broadcast((128, K_DIM, NG, 2, BH)),
                        OP.mult,
                    )
                for ch in range(2):
                    c = 2 * p + ch
                    n = 0
                    for k in range(K_DIM):
                        for g in range(NG):
                            lhs = xc[:, k, g, ch, :]
                            nc.tensor.matmul(
                                ps[0:BH, 16 * c : 16 * (c + 1)],
                                lhs,
                                Wsk[:, k, g, c, :],
                                start=(n == 0),
                                stop=(n == K_DIM * NG - 1),
                            )
                            n += 1

            def tail():
                s_sb = smp.tile(
                    [32, C_CLS, D_DIM], f32, tag="ssb", name=f"srt{j}{h}"
                )
                # exposed tail: evacuate on DVE so the squash chain that
                # follows needs no cross-engine hop
                cp = nc.vector.tensor_copy if exposed else nc.scalar.copy
                cp(
                    s_sb[:].rearrange("b c d -> b (c d)"), state["ps"][0:BH, :]
                )
                v_sb = squash(h, s_sb, last=last)
                if not last:
                    # DVE copies: during an smxc tail ACT's queue is full of
                    # the partner TA's evacuations, so ACT-side copies would
                    # delay the next TA's T-matmuls behind them
                    vts[h] = build_vT(h, v_sb, on_act=False)

            return [head] + [lambda p=p: xc_piece(p) for p in range(5)] + [tail]

        # ---------------- main flow (pass-interleaved emission) ---------
        # TA(j0,h0); TA(j0,h1)+smxc(j0,h0); TA(j1,h0)+smxc(j0,h1);
        # TA(j1,h1)+smxc(j1,h0); smxc(j1,h1)
        vts = {}
        s_sb0 = s0_phase(0)
        v_sb0 = squash(0, s_sb0, last=False)
        vts[0] = build_vT(0, v_sb0)
        s_sb1 = s0_phase(1)
        v_sb1 = squash(1, s_sb1, last=False)
        # vT(h1) is built between the first TA's passes (below) so its
        # transposes don't block the first T-matmuls on the PE queue

        def make_bts(j, h):
            return [
                smp.tile(
                    [128, NG, BH], bf16, tag=f"sm{i}",
                    name=f"bt{j}{h}{i}",
                )
                for i in range(5)
            ]

        def run_plain(fns):
            for f in fns:
                f()

        def run_interleaved(ta_fns, sm_fns):
            # [head, ta0, xc0, ta1, xc1, ta2, xc2, ta3, xc3, ta4, xc4, tail]
            sm_fns[0]()
            for p in range(5):
                ta_fns[p]()
                sm_fns[1 + p]()
            sm_fns[6]()

        bts00 = make_bts(0, 0)
        ta00 = ta_pass_fns(0, 0, *vts[0], bts00)
        ta00[0]()
        vts[1] = build_vT(1, v_sb1, on_act=False)
        for f in ta00[1:]:
            f()
        bts01 = make_bts(0, 1)
        run_interleaved(
            ta_pass_fns(0, 1, *vts[1], bts01),
            smxc_fns(0, 0, bts00, last=False),
        )
        bts10 = make_bts(1, 0)
        run_interleaved(
            ta_pass_fns(1, 0, *vts[0], bts10),
            smxc_fns(0, 1, bts01, last=False),
        )
        bts11 = make_bts(1, 1)
        run_interleaved(
            ta_pass_fns(1, 1, *vts[1], bts11, last_ta=True),
            smxc_fns(1, 0, bts10, last=True),
        )
        run_plain(smxc_fns(1, 1, bts11, last=True, exposed=True))

        for pool in (itp, smp, psV, psT, psS, perm):
            try:
                pool.release()
            except Exception:
                pass

        for pool in (itp, smp, psV, psT, psS, perm):
            try:
                pool.release()
            except Exception:
                pass

    nc.compile()
    return nc


def _consts():
    import ml_dtypes

    return {"eyeb": np.eye(128, dtype=np.float32).astype(ml_dtypes.bfloat16)}


def _prep_w(W0):
    """Host-side layout marshalling of the replicated weights (pure
    permutation + dtype cast; done once, shared by all cores)."""
    import ml_dtypes

    bf = ml_dtypes.bfloat16
    wdt = ml_dtypes.float8_e4m3 if USE_FP8_WT else bf
    ws = WSCALE if USE_FP8_WT else 1.0
    W0 = np.ascontiguousarray(W0, dtype=np.float32)  # [1152, 10, 16, 8]
    wsk_perm = np.ascontiguousarray(
        W0.reshape(NG, 128, C_CLS, D_DIM, K_DIM).transpose(1, 4, 0, 2, 3)
    )  # [128, k, g, c, d]
    wsk = wsk_perm.astype(bf)
    wsk8 = (wsk_perm * ws).astype(wdt)
    wt = np.ascontiguousarray(
        (W0[:, 0:8] * ws).transpose(1, 2, 3, 0).reshape(128, K_DIM, I_CAPS)
    ).astype(wdt)  # rows 16c+d, classes 0-7
    wtb = np.ascontiguousarray(
        (W0[:, 8:10] * ws).transpose(1, 2, 3, 0).reshape(32, K_DIM, I_CAPS)
    ).astype(wdt)  # rows 16(c-8)+d (placed at 96:128 on chip)
    return wsk, wsk8, wt, wtb


def _prep_u(ush):
    import ml_dtypes

    ut = np.ascontiguousarray(
        ush.reshape(B_PER, NG, 128, K_DIM).transpose(2, 3, 1, 0)
    )  # [128, k, g, b]
    u8dt = ml_dtypes.float8_e4m3 if USE_FP8_WT else ml_dtypes.bfloat16
    return ut.astype(ml_dtypes.bfloat16), ut.astype(u8dt)


def get_nc():
    if "nc" not in _CACHE:
        _CACHE["nc"] = _build()
    return _CACHE["nc"]


def make_in_maps(u, W):
    consts = _consts()
    wsk, wsk8, wt, wtb = _prep_w(W[0])
    in_maps = []
    for core in range(N_CORES):
        sh = np.ascontiguousarray(
            u[core * B_PER : (core + 1) * B_PER], dtype=np.float32
        )
        ubf, u8 = _prep_u(sh)
        in_maps.append(
            {
                "uTk_h": ubf,
                "u8_h": u8,
                "wsk_h": wsk,
                "wsk8_h": wsk8,
                "wt_h": wt,
                "wtb_h": wtb,
                **consts,
            }
        )
    return in_maps


def kernel(u: np.ndarray, W: np.ndarray) -> np.ndarray:
    from concourse.bass_utils import run_bass_kernel_spmd

    nc = get_nc()
    in_maps = make_in_maps(u, W)
    res = run_bass_kernel_spmd(nc, in_maps, list(range(N_CORES)))
    out = np.concatenate([res.results[i]["v"] for i in range(N_CORES)], axis=0)
    return out.astype(np.float32)
